# revision 3
# baseline (speedup 1.0000x reference)
"""Additive (Bahdanau) attention on Trainium2, data-parallel over batch on 8 NeuronCores.

Math (per batch b):
    qp = queries @ W_q                     [Tq, H]
    kp = keys @ W_k + b                    [Tk, H]
    scores[q,k] = sum_h v[h] * tanh(qp[q,h] + kp[k,h])
    masked softmax over k (k < seq_len[b]), then out = align @ keys.

Design (per core, 4 batch "slots" with compile-time key-lengths L_slots):
  - host packs keys||ones||maskbias into one "keysx" input so a single DMA per
    k-chunk provides matmul rhs, softmax mask bias and values.
  - keys/queries transposed on PE (identity matmul); projections on PE with the
    b bias folded in via the ones column / W_k||b const rows.
  - kpb duplicated across both 64-partition halves -> kpb2 [128=2h, L].
  - S[h2, j*L+k] = kpb2 + qp2[:, j] per query-pair j via DVE tensor_scalar adds
    (f32 2x port mode), tanh on ACT in two big ops per slot.
  - scores^T[k, q] via PE matmuls: lhsT = tanh tile [128h2, <=128k] stationary,
    rhs = v2blk [128, 2] -> psum [k, 2q] per query pair (block-diagonal v gives
    both queries of a pair in one matmul).
  - exp on ACT from PSUM with per-partition bias = 0/-30000 mask column.
  - final: out_un[q, h] | rowsum = E-chunks (lhsT) @ [keys || ones] (rhs),
    PSUM-accumulated over k-chunks; divide via DVE reciprocal + scale.

Batches are sorted by seq_len and dealt so each core gets one batch per slot
rank; slot k-length = max over the 8 batches of that rank (padded to 8). All
cores run the identical program on different data (SPMD).
"""

import sys

_REPO = "/opt/trn_rl_repo"
if _REPO not in sys.path:
    sys.path.insert(0, _REPO)

import numpy as np

from concourse import bacc, tile
import concourse.mybir as mybir
from concourse import bass_utils

B, TQ, TK, H = 32, 64, 256, 64
NCORES = 8
SLOTS = 4
F32 = mybir.dt.float32
TANH = mybir.ActivationFunctionType.Tanh
EXP = mybir.ActivationFunctionType.Exp
MASK_NEG = -30000.0
KX = H + 2  # keys | ones | maskbias

_prog_cache: dict = {}


def _roundup(x, m):
    return ((x + m - 1) // m) * m


def _chunks(L):
    out, off = [], 0
    while off < L:
        w = min(128, L - off)
        out.append((off, w))
        off += w
    return out


def _build(L_slots):
    nc = bacc.Bacc(
        "TRN2",
        target_bir_lowering=False,
        debug=False,
        enable_asserts=False,
        num_devices=NCORES,
    )
    kx_d = nc.dram_tensor("keysx", [SLOTS, TK, KX], F32, kind="ExternalInput").ap()
    qx_d = nc.dram_tensor("qx", [128, 130], F32, kind="ExternalInput").ap()
    wc_d = nc.dram_tensor("wcomb", [H + 1, 128], F32, kind="ExternalInput").ap()
    id_d = nc.dram_tensor("ident", [128, 128], F32, kind="ExternalInput").ap()
    o_d = nc.dram_tensor("out", [SLOTS, TQ, H], F32, kind="ExternalOutput").ap()

    with tile.TileContext(nc) as tc:
        with (
            tc.tile_pool(name="const", bufs=1) as cpool,
            tc.tile_pool(name="qpool", bufs=1) as qpool,
            tc.tile_pool(name="kpool", bufs=2) as kpool,
            tc.tile_pool(name="wpool", bufs=3) as wpool,
            tc.tile_pool(name="spool", bufs=2) as spool,
            tc.tile_pool(name="tpp", bufs=2, space="PSUM") as tpp,
            tc.tile_pool(name="prj", bufs=2, space="PSUM") as prj,
            tc.tile_pool(name="scp", bufs=3, space="PSUM") as scp,
            tc.tile_pool(name="oup", bufs=1, space="PSUM") as oup,
        ):
            # ---- prefetch everything up front; spread across DMA issuers.
            id_sb = cpool.tile([128, 128], F32, name="id_sb", tag="id")
            nc.sync.dma_start(out=id_sb, in_=id_d)
            qx_sb = cpool.tile([128, 130], F32, name="qx_sb", tag="qx")
            nc.sync.dma_start(out=qx_sb, in_=qx_d)
            wc_sb = cpool.tile([H + 1, 128], F32, name="wc_sb", tag="wc")
            nc.sync.dma_start(out=wc_sb, in_=wc_d)
            v2_sb = qx_sb[:, 128:130]
            wkb_sb = wc_sb[0 : H + 1, 0:H]
            wq_sb = wc_sb[0:H, H:128]

            all_chs = {s: _chunks(L_slots[s]) for s in range(SLOTS)}
            knat = {}
            kdma = 0
            for s in range(SLOTS):
                for ci, (off, w) in enumerate(all_chs[s]):
                    t = kpool.tile(
                        [128, KX], F32, name=f"knat{s}_{ci}", tag=f"knat{s}_{ci}",
                        bufs=1,
                    )
                    eng = nc.sync if kdma < 4 else nc.scalar
                    kdma += 1
                    eng.dma_start(out=t[0:w, :], in_=kx_d[s, off : off + w, :])
                    knat[(s, ci)] = t

            # tiny activation up front so the ACT table set loads early
            scr = cpool.tile([1, 2], F32, name="scr", tag="scr")
            nc.vector.memset(scr, 0.0)
            nc.scalar.activation(scr, scr, TANH)

            # queries: transpose + project, two slots at a time
            qp2g = []
            for g in range(2):
                qT_ps = tpp.tile([H, 128], F32, name=f"qTps{g}", tag="tp")
                nc.tensor.transpose(qT_ps, qx_sb[:, 64 * g : 64 * g + 64], id_sb)
                qT_sb = wpool.tile([H, 128], F32, name=f"qTsb{g}", tag="qT")
                nc.vector.tensor_copy(qT_sb, qT_ps)
                qpT_ps = prj.tile([H, 128], F32, name=f"qpTps{g}", tag="prj")
                nc.tensor.matmul(qpT_ps, lhsT=wq_sb, rhs=qT_sb)
                # qp2[0:64, j] = qpT[:, 2j], qp2[64:128, j] = qpT[:, 2j+1]
                qp2 = qpool.tile([128, 64], F32, name=f"qp2_{g}", tag=f"qp2_{g}")
                nc.vector.tensor_copy(qp2[0:64, :], qpT_ps[:, 0:128:2])
                nc.vector.tensor_copy(qp2[64:128, :], qpT_ps[:, 1:128:2])
                qp2g.append(qp2)

            for s in range(SLOTS):
                L = L_slots[s]
                chs = all_chs[s]
                nch = len(chs)

                keysT = kpool.tile([H + 1, TK], F32, name=f"keysT{s}", tag="keysT")
                for ci, (off, w) in enumerate(chs):
                    kT_ps = tpp.tile([H, 128], F32, name=f"kTps{s}_{ci}", tag="tp")
                    nc.tensor.transpose(
                        kT_ps[0:H, 0:w], knat[(s, ci)][0:w, 0:H], id_sb[0:w, 0:w]
                    )
                    nc.vector.tensor_copy(keysT[0:H, off : off + w], kT_ps[0:H, 0:w])
                nc.gpsimd.memset(keysT[H : H + 1, 0:L], 1.0)

                kpT_ps = prj.tile([H, TK], F32, name=f"kpTps{s}", tag="prj")
                nc.tensor.matmul(kpT_ps[0:H, 0:L], lhsT=wkb_sb, rhs=keysT[:, 0:L])
                kpb2 = wpool.tile([128, TK], F32, name=f"kpb2_{s}", tag="kpb2")
                nc.vector.tensor_copy(kpb2[0:64, 0:L], kpT_ps[0:H, 0:L])
                nc.vector.tensor_copy(kpb2[64:128, 0:L], kpb2[0:64, 0:L])

                qp2 = qp2g[s // 2]
                qoff = 32 * (s % 2)
                S_all = spool.tile([128, 32 * L], F32, name=f"S{s}", tag="S")
                for j in range(32):
                    nc.vector.tensor_scalar_add(
                        S_all[:, j * L : (j + 1) * L],
                        kpb2[:, 0:L],
                        qp2[:, qoff + j : qoff + j + 1],
                    )
                S_tanh = spool.tile([128, 32 * L], F32, name=f"T{s}", tag="T")
                half = 16 * L
                nc.scalar.activation(S_tanh[:, 0:half], S_all[:, 0:half], TANH)
                nc.scalar.activation(
                    S_tanh[:, half : 32 * L], S_all[:, half : 32 * L], TANH
                )

                out_ps = oup.tile([TQ, H + 1], F32, name=f"ops{s}", tag="ou")
                for ci, (off, w) in enumerate(chs):
                    kn = knat[(s, ci)]
                    sc_ps = scp.tile([128, TQ], F32, name=f"sc{s}_{ci}", tag="sc")
                    for j in range(32):
                        nc.tensor.matmul(
                            sc_ps[0:w, 2 * j : 2 * j + 2],
                            lhsT=S_tanh[:, j * L + off : j * L + off + w],
                            rhs=v2_sb,
                            start=True,
                            stop=True,
                        )
                    E = wpool.tile([128, TQ], F32, name=f"E{s}_{ci}", tag=f"E{ci}")
                    nc.scalar.activation(
                        E[0:w, :], sc_ps[0:w, :], EXP, bias=kn[0:w, H + 1 : H + 2]
                    )
                    nc.tensor.matmul(
                        out_ps,
                        lhsT=E[0:w, 0:TQ],
                        rhs=kn[0:w, 0 : H + 1],
                        start=(ci == 0),
                        stop=(ci == nch - 1),
                    )

                recip = wpool.tile([TQ, 1], F32, name=f"rc{s}", tag="rc")
                nc.vector.reciprocal(recip, out_ps[:, H : H + 1])
                out_sb = wpool.tile([TQ, H], F32, name=f"osb{s}", tag="osb")
                nc.vector.tensor_scalar_mul(out_sb, out_ps[:, 0:H], recip)
                nc.gpsimd.dma_start(out=o_d[s], in_=out_sb)

    nc.compile()
    return nc


def _get_prog(L_slots):
    if L_slots not in _prog_cache:
        _prog_cache[L_slots] = _build(L_slots)
    return _prog_cache[L_slots]


def _plan(seq_len_flat):
    sl = np.asarray(seq_len_flat).reshape(-1).astype(np.int64)
    order = np.argsort(-sl, kind="stable")
    assign = np.zeros((NCORES, SLOTS), dtype=np.int64)
    L_slots = []
    for s in range(SLOTS):
        grp = order[NCORES * s : NCORES * (s + 1)]
        assign[:, s] = grp
        L = int(max(1, sl[grp].max()))
        L_slots.append(min(TK, _roundup(L, 8)))
    return tuple(L_slots), assign, sl


def _make_in_maps(queries, keys, sl, assign, W_q, W_k, v, b):
    wcomb = np.zeros((H + 1, 128), np.float32)
    wcomb[0:H, 0:H] = W_k
    wcomb[H, 0:H] = np.asarray(b, np.float32).reshape(-1)
    wcomb[0:H, H:128] = W_q
    ident = np.eye(128, dtype=np.float32)
    vv = np.asarray(v, dtype=np.float32).reshape(-1)

    in_maps = []
    for c in range(NCORES):
        bidx = assign[c]
        qx = np.zeros((128, 130), np.float32)
        qx[:, 0:64] = queries[bidx[0:2]].reshape(128, H)
        qx[:, 64:128] = queries[bidx[2:4]].reshape(128, H)
        qx[0:64, 128] = vv
        qx[64:128, 129] = vv
        keysx = np.zeros((SLOTS, TK, KX), np.float32)
        keysx[:, :, 0:H] = keys[bidx]
        keysx[:, :, H] = 1.0
        for s_i, b_i in enumerate(bidx):
            keysx[s_i, sl[b_i] :, H + 1] = MASK_NEG
        in_maps.append(
            {
                "keysx": keysx,
                "qx": qx,
                "wcomb": wcomb,
                "ident": ident,
            }
        )
    return in_maps


def _run_spmd(nc, in_maps, trace=False, trace_kwargs=None):
    from concourse.bass_interp import get_hw_module

    old = nc.m
    nc.m = get_hw_module(nc.m)
    try:
        res = bass_utils.run_bass_kernel_spmd(
            nc,
            in_maps,
            core_ids=list(range(NCORES)),
            trace=trace,
            **(trace_kwargs or {}),
        )
    finally:
        nc.m = old
    return res


def kernel(queries, keys, seq_len, W_q, W_k, v, b, _trace=False):
    queries = np.asarray(queries, dtype=np.float32)
    keys = np.asarray(keys, dtype=np.float32)
    L_slots, assign, sl = _plan(seq_len)
    nc = _get_prog(L_slots)
    in_maps = _make_in_maps(queries, keys, sl, assign, W_q, W_k, v, b)
    res = _run_spmd(nc, in_maps, trace=_trace)
    out = np.zeros((B, TQ, H), np.float32)
    for c in range(NCORES):
        o = res.results[c]["out"]
        for s_i, b_i in enumerate(assign[c]):
            out[b_i] = o[s_i]
    if _trace:
        kernel._last_results = res
    return out


# revision 4
# speedup vs baseline: 1.8968x; 1.8968x over previous
"""Additive (Bahdanau) attention on Trainium2, data-parallel over batch on 8 NeuronCores.

Math (per batch b):
    qp = queries @ W_q                     [Tq, H]
    kp = keys @ W_k + b                    [Tk, H]
    scores[q,k] = sum_h v[h] * tanh(qp[q,h] + kp[k,h])
    masked softmax over k (k < seq_len[b]), then out = align @ keys.

Design (per core, 4 batch "slots" with compile-time key-lengths L_slots):
  - host packs keys||ones||maskbias into one "keysx" input so a single DMA per
    k-chunk provides matmul rhs, softmax mask bias and values.
  - keys/queries transposed on PE (identity matmul); projections on PE with the
    b bias folded in via the ones column / W_k||b const rows.
  - kpb duplicated across both 64-partition halves -> kpb2 [128=2h, L].
  - S[h2, j*L+k] = kpb2 + qp2[:, j] per query-pair j via DVE tensor_scalar adds
    (f32 2x port mode), tanh on ACT in two big ops per slot.
  - scores^T[k, q] via PE matmuls: lhsT = tanh tile [128h2, <=128k] stationary,
    rhs = v2blk [128, 2] -> psum [k, 2q] per query pair (block-diagonal v gives
    both queries of a pair in one matmul).
  - exp on ACT from PSUM with per-partition bias = 0/-30000 mask column.
  - final: out_un[q, h] | rowsum = E-chunks (lhsT) @ [keys || ones] (rhs),
    PSUM-accumulated over k-chunks; divide via DVE reciprocal + scale.

Batches are sorted by seq_len and dealt so each core gets one batch per slot
rank; slot k-length = max over the 8 batches of that rank (padded to 8). All
cores run the identical program on different data (SPMD).
"""

import sys

_REPO = "/opt/trn_rl_repo"
if _REPO not in sys.path:
    sys.path.insert(0, _REPO)

import numpy as np

from concourse import bacc, tile
import concourse.mybir as mybir
from concourse import bass_utils

B, TQ, TK, H = 32, 64, 256, 64
NCORES = 8
SLOTS = 4
F32 = mybir.dt.float32
BF16 = mybir.dt.bfloat16
TANH = mybir.ActivationFunctionType.Tanh
EXP = mybir.ActivationFunctionType.Exp
MASK_NEG = -30000.0
KX = H + 2  # keys | ones | maskbias

_prog_cache: dict = {}


def _roundup(x, m):
    return ((x + m - 1) // m) * m


def _chunks(L):
    out, off = [], 0
    while off < L:
        w = min(128, L - off)
        out.append((off, w))
        off += w
    return out


def _build(L_slots):
    nc = bacc.Bacc(
        "TRN2",
        target_bir_lowering=False,
        debug=False,
        enable_asserts=False,
        num_devices=NCORES,
    )
    kx_d = nc.dram_tensor("keysx", [SLOTS, TK, KX], F32, kind="ExternalInput").ap()
    qx_d = nc.dram_tensor("qx", [128, 130], F32, kind="ExternalInput").ap()
    wc_d = nc.dram_tensor("wcomb", [H, 132], F32, kind="ExternalInput").ap()
    id_d = nc.dram_tensor("ident", [128, 128], F32, kind="ExternalInput").ap()
    o_d = nc.dram_tensor("out", [SLOTS, TQ, H], F32, kind="ExternalOutput").ap()

    with tile.TileContext(nc) as tc:
        with (
            tc.tile_pool(name="const", bufs=1) as cpool,
            tc.tile_pool(name="qpool", bufs=1) as qpool,
            tc.tile_pool(name="kpool", bufs=2) as kpool,
            tc.tile_pool(name="wpool", bufs=3) as wpool,
            tc.tile_pool(name="spool", bufs=2) as spool,
            tc.tile_pool(name="tpp", bufs=2, space="PSUM") as tpp,
            tc.tile_pool(name="prj", bufs=2, space="PSUM") as prj,
            tc.tile_pool(name="scp", bufs=3, space="PSUM") as scp,
            tc.tile_pool(name="oup", bufs=1, space="PSUM") as oup,
        ):
            # ---- prefetch everything up front; spread across DMA issuers.
            id_sb = cpool.tile([128, 128], F32, name="id_sb", tag="id")
            nc.sync.dma_start(out=id_sb, in_=id_d)
            qx_sb = cpool.tile([128, 130], F32, name="qx_sb", tag="qx")
            nc.sync.dma_start(out=qx_sb, in_=qx_d)
            wc_sb = cpool.tile([H, 132], F32, name="wc_sb", tag="wc")
            nc.sync.dma_start(out=wc_sb, in_=wc_d)
            wk_sb = wc_sb[0:H, 0:H]
            wq_sb = wc_sb[0:H, H : 2 * H]
            b_col = wc_sb[0:H, 2 * H : 2 * H + 1]
            v2_sb = cpool.tile([128, 2], BF16, name="v2_sb", tag="v2")

            all_chs = {s: _chunks(L_slots[s]) for s in range(SLOTS)}
            knat = {}
            kdma = 0
            for s in range(SLOTS):
                for ci, (off, w) in enumerate(all_chs[s]):
                    t = kpool.tile(
                        [128, KX], F32, name=f"knat{s}_{ci}", tag=f"knat{s}_{ci}",
                        bufs=1,
                    )
                    eng = nc.sync if kdma < 4 else nc.scalar
                    kdma += 1
                    eng.dma_start(out=t[0:w, :], in_=kx_d[s, off : off + w, :])
                    knat[(s, ci)] = t

            # tiny activation up front so the ACT table set loads early
            scr = cpool.tile([1, 2], F32, name="scr", tag="scr")
            nc.vector.memset(scr, 0.0)
            nc.scalar.activation(scr, scr, TANH)
            nc.vector.tensor_copy(v2_sb, qx_sb[:, 128:130])

            # queries: transpose + project, two slots at a time
            qp2g = []
            for g in range(2):
                qT_ps = tpp.tile([H, 128], F32, name=f"qTps{g}", tag="tp")
                nc.tensor.transpose(qT_ps, qx_sb[:, 64 * g : 64 * g + 64], id_sb)
                qT_sb = wpool.tile([H, 128], F32, name=f"qTsb{g}", tag="qT")
                nc.vector.tensor_copy(qT_sb, qT_ps)
                qpT_ps = prj.tile([H, 128], F32, name=f"qpTps{g}", tag="prj")
                nc.tensor.matmul(qpT_ps, lhsT=wq_sb, rhs=qT_sb)
                # qp2rep[0:64, 2j] = qp2rep[0:64, 2j+1] = qpT[:, 2j]
                # qp2rep[64:128, 2j] = qp2rep[64:128, 2j+1] = qpT[:, 2j+1]
                qp2 = qpool.tile([128, 128], BF16, name=f"qp2_{g}", tag=f"qp2_{g}")
                nc.vector.tensor_copy(qp2[0:64, 0:128:2], qpT_ps[:, 0:128:2])
                nc.vector.tensor_copy(qp2[0:64, 1:128:2], qpT_ps[:, 0:128:2])
                nc.vector.tensor_copy(qp2[64:128, 0:128:2], qpT_ps[:, 1:128:2])
                nc.vector.tensor_copy(qp2[64:128, 1:128:2], qpT_ps[:, 1:128:2])
                qp2g.append(qp2)

            for s in range(SLOTS):
                L = L_slots[s]
                chs = all_chs[s]
                nch = len(chs)

                keysT = kpool.tile([H, TK], F32, name=f"keysT{s}", tag="keysT")
                for ci, (off, w) in enumerate(chs):
                    kT_ps = tpp.tile([H, 128], F32, name=f"kTps{s}_{ci}", tag="tp")
                    nc.tensor.transpose(
                        kT_ps[0:H, 0:w], knat[(s, ci)][0:w, 0:H], id_sb[0:w, 0:w]
                    )
                    nc.vector.tensor_copy(keysT[0:H, off : off + w], kT_ps[0:H, 0:w])
                kpT_ps = prj.tile([H, TK], F32, name=f"kpTps{s}", tag="prj")
                nc.tensor.matmul(kpT_ps[0:H, 0:L], lhsT=wk_sb, rhs=keysT[:, 0:L])
                kpb2 = wpool.tile([128, TK], BF16, name=f"kpb2_{s}", tag="kpb2")
                nc.vector.tensor_scalar_add(kpb2[0:64, 0:L], kpT_ps[0:H, 0:L], b_col)
                nc.vector.tensor_scalar_add(kpb2[64:128, 0:L], kpT_ps[0:H, 0:L], b_col)

                qp2 = qp2g[s // 2]
                qoff = 64 * (s % 2)
                S_all = spool.tile([128, 32 * L], BF16, name=f"S{s}", tag="S")
                in0 = (
                    kpb2[:, 0:L]
                    .rearrange("c (k two) -> c k two", two=2)
                    .unsqueeze(1)
                    .broadcast_to([128, 32, L // 2, 2])
                )
                in1 = (
                    qp2[:, qoff : qoff + 64]
                    .rearrange("c (j two) -> c j two", two=2)
                    .unsqueeze(2)
                    .broadcast_to([128, 32, L // 2, 2])
                )
                s_out = S_all.rearrange(
                    "c (j k two) -> c j k two", two=2, k=L // 2
                )
                nc.vector.tensor_add(s_out, in0, in1)
                S_tanh = spool.tile([128, 32 * L], BF16, name=f"T{s}", tag="T")
                half = 16 * L
                nc.scalar.activation(S_tanh[:, 0:half], S_all[:, 0:half], TANH)
                nc.scalar.activation(
                    S_tanh[:, half : 32 * L], S_all[:, half : 32 * L], TANH
                )

                out_ps = oup.tile([TQ, H + 1], F32, name=f"ops{s}", tag="ou")
                for ci, (off, w) in enumerate(chs):
                    kn = knat[(s, ci)]
                    sc_ps = scp.tile([128, TQ], F32, name=f"sc{s}_{ci}", tag="sc")
                    for j in range(32):
                        nc.tensor.matmul(
                            sc_ps[0:w, 2 * j : 2 * j + 2],
                            lhsT=S_tanh[:, j * L + off : j * L + off + w],
                            rhs=v2_sb,
                            start=True,
                            stop=True,
                        )
                    E = wpool.tile([128, TQ], F32, name=f"E{s}_{ci}", tag=f"E{ci}")
                    nc.scalar.activation(
                        E[0:w, :], sc_ps[0:w, :], EXP, bias=kn[0:w, H + 1 : H + 2]
                    )
                    nc.tensor.matmul(
                        out_ps,
                        lhsT=E[0:w, 0:TQ],
                        rhs=kn[0:w, 0 : H + 1],
                        start=(ci == 0),
                        stop=(ci == nch - 1),
                    )

                recip = wpool.tile([TQ, 1], F32, name=f"rc{s}", tag="rc")
                nc.vector.reciprocal(recip, out_ps[:, H : H + 1])
                out_sb = wpool.tile([TQ, H], F32, name=f"osb{s}", tag="osb")
                nc.vector.tensor_scalar_mul(out_sb, out_ps[:, 0:H], recip)
                nc.sync.dma_start(out=o_d[s], in_=out_sb)

    nc.compile()
    return nc


def _get_prog(L_slots):
    if L_slots not in _prog_cache:
        _prog_cache[L_slots] = _build(L_slots)
    return _prog_cache[L_slots]


def _plan(seq_len_flat):
    sl = np.asarray(seq_len_flat).reshape(-1).astype(np.int64)
    order = np.argsort(-sl, kind="stable")
    assign = np.zeros((NCORES, SLOTS), dtype=np.int64)
    L_slots = []
    for s in range(SLOTS):
        grp = order[NCORES * s : NCORES * (s + 1)]
        assign[:, s] = grp
        L = int(max(1, sl[grp].max()))
        L_slots.append(min(TK, _roundup(L, 8)))
    return tuple(L_slots), assign, sl


def _make_in_maps(queries, keys, sl, assign, W_q, W_k, v, b):
    wcomb = np.zeros((H, 132), np.float32)
    wcomb[0:H, 0:H] = W_k
    wcomb[0:H, H : 2 * H] = W_q
    wcomb[0:H, 2 * H] = np.asarray(b, np.float32).reshape(-1)
    ident = np.eye(128, dtype=np.float32)
    vv = np.asarray(v, dtype=np.float32).reshape(-1)

    in_maps = []
    for c in range(NCORES):
        bidx = assign[c]
        qx = np.zeros((128, 130), np.float32)
        qx[:, 0:64] = queries[bidx[0:2]].reshape(128, H)
        qx[:, 64:128] = queries[bidx[2:4]].reshape(128, H)
        qx[0:64, 128] = vv
        qx[64:128, 129] = vv
        keysx = np.zeros((SLOTS, TK, KX), np.float32)
        keysx[:, :, 0:H] = keys[bidx]
        keysx[:, :, H] = 1.0
        for s_i, b_i in enumerate(bidx):
            keysx[s_i, sl[b_i] :, H + 1] = MASK_NEG
        in_maps.append(
            {
                "keysx": keysx,
                "qx": qx,
                "wcomb": wcomb,
                "ident": ident,
            }
        )
    return in_maps


def _run_spmd(nc, in_maps, trace=False, trace_kwargs=None):
    from concourse.bass_interp import get_hw_module

    old = nc.m
    nc.m = get_hw_module(nc.m)
    try:
        res = bass_utils.run_bass_kernel_spmd(
            nc,
            in_maps,
            core_ids=list(range(NCORES)),
            trace=trace,
            **(trace_kwargs or {}),
        )
    finally:
        nc.m = old
    return res


def kernel(queries, keys, seq_len, W_q, W_k, v, b, _trace=False):
    queries = np.asarray(queries, dtype=np.float32)
    keys = np.asarray(keys, dtype=np.float32)
    L_slots, assign, sl = _plan(seq_len)
    nc = _get_prog(L_slots)
    in_maps = _make_in_maps(queries, keys, sl, assign, W_q, W_k, v, b)
    res = _run_spmd(nc, in_maps, trace=_trace)
    out = np.zeros((B, TQ, H), np.float32)
    for c in range(NCORES):
        o = res.results[c]["out"]
        for s_i, b_i in enumerate(assign[c]):
            out[b_i] = o[s_i]
    if _trace:
        kernel._last_results = res
    return out


# revision 6
# speedup vs baseline: 2.0064x; 1.0578x over previous
"""Additive (Bahdanau) attention on Trainium2, data-parallel over batch on 8 NeuronCores.

Math (per batch b):
    qp = queries @ W_q                     [Tq, H]
    kp = keys @ W_k + b                    [Tk, H]
    scores[q,k] = sum_h v[h] * tanh(qp[q,h] + kp[k,h])
    masked softmax over k (k < seq_len[b]), then out = align @ keys.

Design (per core, 4 batch "slots" with compile-time key-lengths L_slots):
  - host packs keys||ones||maskbias into one "keysx" input so a single DMA per
    k-chunk provides matmul rhs, softmax mask bias and values.
  - keys/queries transposed on PE (identity matmul); projections on PE with the
    b bias folded in via the ones column / W_k||b const rows.
  - kpb duplicated across both 64-partition halves -> kpb2 [128=2h, L].
  - S[h2, j*L+k] = kpb2 + qp2[:, j] per query-pair j via DVE tensor_scalar adds
    (f32 2x port mode), tanh on ACT in two big ops per slot.
  - scores^T[k, q] via PE matmuls: lhsT = tanh tile [128h2, <=128k] stationary,
    rhs = v2blk [128, 2] -> psum [k, 2q] per query pair (block-diagonal v gives
    both queries of a pair in one matmul).
  - exp on ACT from PSUM with per-partition bias = 0/-30000 mask column.
  - final: out_un[q, h] | rowsum = E-chunks (lhsT) @ [keys || ones] (rhs),
    PSUM-accumulated over k-chunks; divide via DVE reciprocal + scale.

Batches are sorted by seq_len and dealt so each core gets one batch per slot
rank; slot k-length = max over the 8 batches of that rank (padded to 8). All
cores run the identical program on different data (SPMD).
"""

import sys

_REPO = "/opt/trn_rl_repo"
if _REPO not in sys.path:
    sys.path.insert(0, _REPO)

import numpy as np

from concourse import bacc, tile
import concourse.mybir as mybir
from concourse import bass_utils

B, TQ, TK, H = 32, 64, 256, 64
NCORES = 8
SLOTS = 4
F32 = mybir.dt.float32
BF16 = mybir.dt.bfloat16
TANH = mybir.ActivationFunctionType.Tanh
EXP = mybir.ActivationFunctionType.Exp
MASK_NEG = -30000.0
KX = H + 2  # keys | ones | maskbias

_prog_cache: dict = {}


def _roundup(x, m):
    return ((x + m - 1) // m) * m


def _chunks(L):
    out, off = [], 0
    while off < L:
        w = min(128, L - off)
        out.append((off, w))
        off += w
    return out


def _build(L_slots):
    nc = bacc.Bacc(
        "TRN2",
        target_bir_lowering=False,
        debug=False,
        enable_asserts=False,
        num_devices=NCORES,
    )
    kx_d = nc.dram_tensor("keysx", [SLOTS, TK, KX], F32, kind="ExternalInput").ap()
    cx_d = nc.dram_tensor("cx", [128, 390], F32, kind="ExternalInput").ap()
    o_d = nc.dram_tensor("out", [SLOTS, TQ, H], F32, kind="ExternalOutput").ap()

    with tile.TileContext(nc) as tc:
        with (
            tc.tile_pool(name="const", bufs=1) as cpool,
            tc.tile_pool(name="qpool", bufs=1) as qpool,
            tc.tile_pool(name="kpool", bufs=2) as kpool,
            tc.tile_pool(name="wpool", bufs=3) as wpool,
            tc.tile_pool(name="spool", bufs=2) as spool,
            tc.tile_pool(name="tpp", bufs=2, space="PSUM") as tpp,
            tc.tile_pool(name="prj", bufs=2, space="PSUM") as prj,
            tc.tile_pool(name="scp", bufs=3, space="PSUM") as scp,
            tc.tile_pool(name="oup", bufs=1, space="PSUM") as oup,
        ):
            # tiny activation up front so the ACT table set loads early
            scr = cpool.tile([1, 2], F32, name="scr", tag="scr")
            nc.vector.memset(scr, 0.0)
            nc.scalar.activation(scr, scr, TANH)

            # ---- prefetch everything up front in two big DMAs.
            cx_sb = cpool.tile([128, 390], F32, name="cx_sb", tag="cx")
            nc.sync.dma_start(out=cx_sb, in_=cx_d)
            qx_sb = cx_sb[:, 0:130]
            id_sb = cx_sb[:, 130:258]
            wk_sb = cx_sb[0:H, 258:322]
            wq_sb = cx_sb[0:H, 322:386]
            b_col = cx_sb[0:H, 386:387]
            v2_sb = cpool.tile([128, 2], BF16, name="v2_sb", tag="v2")
            nc.vector.tensor_copy(v2_sb, qx_sb[:, 128:130])

            all_chs = {s: _chunks(L_slots[s]) for s in range(SLOTS)}
            knat_all = kpool.tile(
                [128, 8 * KX], F32, name="knat_all", tag="knat_all", bufs=1
            )
            nc.sync.dma_start(
                out=knat_all.rearrange("p (sc x) -> p sc x", x=KX),
                in_=kx_d.rearrange("s (c p) x -> p (s c) x", p=128),
            )
            knat = {}
            for s in range(SLOTS):
                for ci, (off, w) in enumerate(all_chs[s]):
                    idx = 2 * s + ci
                    knat[(s, ci)] = knat_all[:, idx * KX : (idx + 1) * KX]

            # queries: transpose + project, two slots at a time
            qp2g = []
            for g in range(2):
                qT_ps = tpp.tile([H, 128], F32, name=f"qTps{g}", tag="tp")
                nc.tensor.transpose(qT_ps, qx_sb[:, 64 * g : 64 * g + 64], id_sb)
                qT_sb = wpool.tile([H, 128], F32, name=f"qTsb{g}", tag="qT")
                nc.vector.tensor_copy(qT_sb, qT_ps)
                qpT_ps = prj.tile([H, 128], F32, name=f"qpTps{g}", tag="prj")
                nc.tensor.matmul(qpT_ps, lhsT=wq_sb, rhs=qT_sb)
                # qp2rep[0:64, 2j] = qp2rep[0:64, 2j+1] = qpT[:, 2j]
                # qp2rep[64:128, 2j] = qp2rep[64:128, 2j+1] = qpT[:, 2j+1]
                qp2 = qpool.tile([128, 128], BF16, name=f"qp2_{g}", tag=f"qp2_{g}")
                nc.vector.tensor_copy(qp2[0:64, 0:128:2], qpT_ps[:, 0:128:2])
                nc.vector.tensor_copy(qp2[0:64, 1:128:2], qpT_ps[:, 0:128:2])
                nc.vector.tensor_copy(qp2[64:128, 0:128:2], qpT_ps[:, 1:128:2])
                nc.vector.tensor_copy(qp2[64:128, 1:128:2], qpT_ps[:, 1:128:2])
                qp2g.append(qp2)

            for s in range(SLOTS):
                L = L_slots[s]
                chs = all_chs[s]
                nch = len(chs)

                keysT = kpool.tile([H, TK], F32, name=f"keysT{s}", tag="keysT")
                for ci, (off, w) in enumerate(chs):
                    kT_ps = tpp.tile([H, 128], F32, name=f"kTps{s}_{ci}", tag="tp")
                    nc.tensor.transpose(
                        kT_ps[0:H, 0:w], knat[(s, ci)][0:w, 0:H], id_sb[0:w, 0:w]
                    )
                    nc.vector.tensor_copy(keysT[0:H, off : off + w], kT_ps[0:H, 0:w])
                kpT_ps = prj.tile([H, TK], F32, name=f"kpTps{s}", tag="prj")
                nc.tensor.matmul(kpT_ps[0:H, 0:L], lhsT=wk_sb, rhs=keysT[:, 0:L])
                kpb2 = wpool.tile([128, TK], BF16, name=f"kpb2_{s}", tag="kpb2")
                nc.vector.tensor_scalar_add(kpb2[0:64, 0:L], kpT_ps[0:H, 0:L], b_col)
                nc.vector.tensor_scalar_add(kpb2[64:128, 0:L], kpT_ps[0:H, 0:L], b_col)

                qp2 = qp2g[s // 2]
                qoff = 64 * (s % 2)
                S_all = spool.tile([128, 32 * L], BF16, name=f"S{s}", tag="S")
                in0 = (
                    kpb2[:, 0:L]
                    .rearrange("c (k two) -> c k two", two=2)
                    .unsqueeze(1)
                    .broadcast_to([128, 16, L // 2, 2])
                )
                for hh in range(2):
                    in1 = (
                        qp2[:, qoff + 32 * hh : qoff + 32 * hh + 32]
                        .rearrange("c (j two) -> c j two", two=2)
                        .unsqueeze(2)
                        .broadcast_to([128, 16, L // 2, 2])
                    )
                    s_out = S_all[:, 16 * L * hh : 16 * L * (hh + 1)].rearrange(
                        "c (j k two) -> c j k two", two=2, k=L // 2
                    )
                    nc.vector.tensor_add(s_out, in0, in1)
                S_tanh = spool.tile([128, 32 * L], BF16, name=f"T{s}", tag="T")
                half = 16 * L
                nc.scalar.activation(S_tanh[:, 0:half], S_all[:, 0:half], TANH)
                nc.scalar.activation(
                    S_tanh[:, half : 32 * L], S_all[:, half : 32 * L], TANH
                )

                out_ps = oup.tile([TQ, H + 1], F32, name=f"ops{s}", tag="ou")
                for ci, (off, w) in enumerate(chs):
                    kn = knat[(s, ci)]
                    sc_ps = scp.tile([128, TQ], F32, name=f"sc{s}_{ci}", tag="sc")
                    for j in range(32):
                        nc.tensor.matmul(
                            sc_ps[0:w, 2 * j : 2 * j + 2],
                            lhsT=S_tanh[:, j * L + off : j * L + off + w],
                            rhs=v2_sb,
                            start=True,
                            stop=True,
                        )
                    E = wpool.tile([128, TQ], F32, name=f"E{s}_{ci}", tag=f"E{ci}")
                    nc.scalar.activation(
                        E[0:w, :], sc_ps[0:w, :], EXP, bias=kn[0:w, H + 1 : H + 2]
                    )
                    nc.tensor.matmul(
                        out_ps,
                        lhsT=E[0:w, 0:TQ],
                        rhs=kn[0:w, 0 : H + 1],
                        start=(ci == 0),
                        stop=(ci == nch - 1),
                    )

                recip = wpool.tile([TQ, 1], F32, name=f"rc{s}", tag="rc")
                nc.vector.reciprocal(recip, out_ps[:, H : H + 1])
                out_sb = wpool.tile([TQ, H], F32, name=f"osb{s}", tag="osb")
                nc.vector.tensor_scalar_mul(out_sb, out_ps[:, 0:H], recip)
                nc.sync.dma_start(out=o_d[s], in_=out_sb)

    nc.compile()
    return nc


def _get_prog(L_slots):
    if L_slots not in _prog_cache:
        _prog_cache[L_slots] = _build(L_slots)
    return _prog_cache[L_slots]


def _plan(seq_len_flat):
    sl = np.asarray(seq_len_flat).reshape(-1).astype(np.int64)
    order = np.argsort(-sl, kind="stable")
    assign = np.zeros((NCORES, SLOTS), dtype=np.int64)
    L_slots = []
    for s in range(SLOTS):
        grp = order[NCORES * s : NCORES * (s + 1)]
        assign[:, s] = grp
        L = int(max(1, sl[grp].max()))
        L_slots.append(min(TK, _roundup(L, 8)))
    return tuple(L_slots), assign, sl


def _make_in_maps(queries, keys, sl, assign, W_q, W_k, v, b):
    vv = np.asarray(v, dtype=np.float32).reshape(-1)
    base = np.zeros((128, 390), np.float32)
    base[:, 130:258] = np.eye(128, dtype=np.float32)
    base[0:H, 258:322] = W_k
    base[0:H, 322:386] = W_q
    base[0:H, 386] = np.asarray(b, np.float32).reshape(-1)

    in_maps = []
    for c in range(NCORES):
        bidx = assign[c]
        cx = base.copy()
        cx[:, 0:64] = queries[bidx[0:2]].reshape(128, H)
        cx[:, 64:128] = queries[bidx[2:4]].reshape(128, H)
        cx[0:64, 128] = vv
        cx[64:128, 129] = vv
        keysx = np.zeros((SLOTS, TK, KX), np.float32)
        keysx[:, :, 0:H] = keys[bidx]
        keysx[:, :, H] = 1.0
        for s_i, b_i in enumerate(bidx):
            keysx[s_i, sl[b_i] :, H + 1] = MASK_NEG
        in_maps.append(
            {
                "keysx": keysx,
                "cx": cx,
            }
        )
    return in_maps


def _run_spmd(nc, in_maps, trace=False, trace_kwargs=None):
    from concourse.bass_interp import get_hw_module

    old = nc.m
    nc.m = get_hw_module(nc.m)
    try:
        res = bass_utils.run_bass_kernel_spmd(
            nc,
            in_maps,
            core_ids=list(range(NCORES)),
            trace=trace,
            **(trace_kwargs or {}),
        )
    finally:
        nc.m = old
    return res


def kernel(queries, keys, seq_len, W_q, W_k, v, b, _trace=False):
    queries = np.asarray(queries, dtype=np.float32)
    keys = np.asarray(keys, dtype=np.float32)
    L_slots, assign, sl = _plan(seq_len)
    nc = _get_prog(L_slots)
    in_maps = _make_in_maps(queries, keys, sl, assign, W_q, W_k, v, b)
    res = _run_spmd(nc, in_maps, trace=_trace)
    out = np.zeros((B, TQ, H), np.float32)
    for c in range(NCORES):
        o = res.results[c]["out"]
        for s_i, b_i in enumerate(assign[c]):
            out[b_i] = o[s_i]
    if _trace:
        kernel._last_results = res
    return out


# revision 8
# speedup vs baseline: 2.2075x; 1.1002x over previous
"""Additive (Bahdanau) attention on Trainium2, data-parallel over batch on 8 NeuronCores.

Math (per batch b):
    qp = queries @ W_q                     [Tq, H]
    kp = keys @ W_k + b                    [Tk, H]
    scores[q,k] = sum_h v[h] * tanh(qp[q,h] + kp[k,h])
    masked softmax over k (k < seq_len[b]), then out = align @ keys.

Design (per core, 4 batch "slots" with compile-time key-lengths L_slots):
  - host packs keys||ones||maskbias into one "keysx" input so a single DMA per
    k-chunk provides matmul rhs, softmax mask bias and values.
  - keys/queries transposed on PE (identity matmul); projections on PE with the
    b bias folded in via the ones column / W_k||b const rows.
  - kpb duplicated across both 64-partition halves -> kpb2 [128=2h, L].
  - S[h2, j*L+k] = kpb2 + qp2[:, j] per query-pair j via DVE tensor_scalar adds
    (f32 2x port mode), tanh on ACT in two big ops per slot.
  - scores^T[k, q] via PE matmuls: lhsT = tanh tile [128h2, <=128k] stationary,
    rhs = v2blk [128, 2] -> psum [k, 2q] per query pair (block-diagonal v gives
    both queries of a pair in one matmul).
  - exp on ACT from PSUM with per-partition bias = 0/-30000 mask column.
  - final: out_un[q, h] | rowsum = E-chunks (lhsT) @ [keys || ones] (rhs),
    PSUM-accumulated over k-chunks; divide via DVE reciprocal + scale.

Batches are sorted by seq_len and dealt so each core gets one batch per slot
rank; slot k-length = max over the 8 batches of that rank (padded to 8). All
cores run the identical program on different data (SPMD).
"""

import sys

_REPO = "/opt/trn_rl_repo"
if _REPO not in sys.path:
    sys.path.insert(0, _REPO)

import numpy as np

from concourse import bacc, tile
import concourse.mybir as mybir
from concourse import bass_utils

B, TQ, TK, H = 32, 64, 256, 64
NCORES = 8
SLOTS = 4
F32 = mybir.dt.float32
BF16 = mybir.dt.bfloat16
TANH = mybir.ActivationFunctionType.Tanh
EXP = mybir.ActivationFunctionType.Exp
MASK_NEG = -30000.0
KX = H + 2  # keys | ones | maskbias

_prog_cache: dict = {}


def _roundup(x, m):
    return ((x + m - 1) // m) * m


def _chunks(L):
    out, off = [], 0
    while off < L:
        w = min(128, L - off)
        out.append((off, w))
        off += w
    return out


def _build(L_slots):
    nc = bacc.Bacc(
        "TRN2",
        target_bir_lowering=False,
        debug=False,
        enable_asserts=False,
        num_devices=NCORES,
    )
    kx_d = nc.dram_tensor("keysx", [SLOTS, TK, KX], F32, kind="ExternalInput").ap()
    cx_d = nc.dram_tensor("cx", [128, 390], F32, kind="ExternalInput").ap()
    o_d = nc.dram_tensor("out", [SLOTS, TQ, H], F32, kind="ExternalOutput").ap()

    with tile.TileContext(nc) as tc:
        with (
            tc.tile_pool(name="const", bufs=1) as cpool,
            tc.tile_pool(name="qpool", bufs=1) as qpool,
            tc.tile_pool(name="kpool", bufs=2) as kpool,
            tc.tile_pool(name="wpool", bufs=3) as wpool,
            tc.tile_pool(name="spool", bufs=2) as spool,
            tc.tile_pool(name="tpp", bufs=2, space="PSUM") as tpp,
            tc.tile_pool(name="prj", bufs=2, space="PSUM") as prj,
            tc.tile_pool(name="scp", bufs=2, space="PSUM") as scp,
            tc.tile_pool(name="oup", bufs=2, space="PSUM") as oup,
        ):
            # tiny activation up front so the ACT table set loads early
            scr = cpool.tile([1, 2], F32, name="scr", tag="scr")
            nc.vector.memset(scr, 0.0)
            nc.scalar.activation(scr, scr, TANH)

            # ---- prefetch everything up front in two big DMAs.
            cx_sb = cpool.tile([128, 390], F32, name="cx_sb", tag="cx")
            nc.sync.dma_start(out=cx_sb, in_=cx_d)
            qx_sb = cx_sb[:, 0:130]
            id_sb = cx_sb[:, 130:258]
            wk_sb = cx_sb[0:H, 258:322]
            wq_sb = cx_sb[0:H, 322:386]
            b_col = cx_sb[0:H, 386:387]
            v2_sb = cpool.tile([128, 2], BF16, name="v2_sb", tag="v2")
            nc.vector.tensor_copy(v2_sb, qx_sb[:, 128:130])

            all_chs = {s: _chunks(L_slots[s]) for s in range(SLOTS)}
            knat_all = kpool.tile(
                [128, 8 * KX], F32, name="knat_all", tag="knat_all", bufs=1
            )
            nc.sync.dma_start(
                out=knat_all.rearrange("p (sc x) -> p sc x", x=KX),
                in_=kx_d.rearrange("s (c p) x -> p (s c) x", p=128),
            )
            knat = {}
            for s in range(SLOTS):
                for ci, (off, w) in enumerate(all_chs[s]):
                    idx = 2 * s + ci
                    knat[(s, ci)] = knat_all[:, idx * KX : (idx + 1) * KX]

            # queries: transpose + project, two slots at a time
            qp2g = []
            for g in range(2):
                qT_ps = tpp.tile([H, 128], F32, name=f"qTps{g}", tag="tp")
                nc.tensor.transpose(qT_ps, qx_sb[:, 64 * g : 64 * g + 64], id_sb)
                qT_sb = wpool.tile([H, 128], F32, name=f"qTsb{g}", tag="qT")
                nc.scalar.copy(qT_sb, qT_ps)
                qpT_ps = prj.tile([H, 128], F32, name=f"qpTps{g}", tag="prj")
                nc.tensor.matmul(qpT_ps, lhsT=wq_sb, rhs=qT_sb)
                # qp2rep[0:64, 2j] = qp2rep[0:64, 2j+1] = qpT[:, 2j]
                # qp2rep[64:128, 2j] = qp2rep[64:128, 2j+1] = qpT[:, 2j+1]
                qp2 = qpool.tile([128, 128], BF16, name=f"qp2_{g}", tag=f"qp2_{g}")
                nc.vector.tensor_copy(
                    qp2[0:64, :].rearrange("c (j two) -> c j two", two=2),
                    qpT_ps[:, 0:128:2].unsqueeze(2).broadcast_to([H, 64, 2]),
                )
                nc.vector.tensor_copy(
                    qp2[64:128, :].rearrange("c (j two) -> c j two", two=2),
                    qpT_ps[:, 1:128:2].unsqueeze(2).broadcast_to([H, 64, 2]),
                )
                qp2g.append(qp2)

            for s in range(SLOTS):
                L = L_slots[s]
                chs = all_chs[s]
                nch = len(chs)

                keysT = kpool.tile([H, TK], F32, name=f"keysT{s}", tag="keysT")
                for ci, (off, w) in enumerate(chs):
                    kT_ps = tpp.tile([H, 128], F32, name=f"kTps{s}_{ci}", tag="tp")
                    nc.tensor.transpose(
                        kT_ps[0:H, 0:w], knat[(s, ci)][0:w, 0:H], id_sb[0:w, 0:w]
                    )
                    if s < 2:
                        nc.scalar.copy(keysT[0:H, off : off + w], kT_ps[0:H, 0:w])
                    else:
                        nc.vector.tensor_copy(keysT[0:H, off : off + w], kT_ps[0:H, 0:w])
                kpT_ps = prj.tile([H, TK], F32, name=f"kpTps{s}", tag="prj")
                nc.tensor.matmul(kpT_ps[0:H, 0:L], lhsT=wk_sb, rhs=keysT[:, 0:L])
                kpb2 = wpool.tile([128, TK], BF16, name=f"kpb2_{s}", tag="kpb2")
                nc.vector.tensor_scalar_add(kpb2[0:64, 0:L], kpT_ps[0:H, 0:L], b_col)
                nc.vector.tensor_scalar_add(kpb2[64:128, 0:L], kpT_ps[0:H, 0:L], b_col)

                qp2 = qp2g[s // 2]
                qoff = 64 * (s % 2)
                S_all = spool.tile([128, 32 * L], BF16, name=f"S{s}", tag="S")
                for j0, j1 in ((0, 8), (8, 32)):
                    nj = j1 - j0
                    in0 = (
                        kpb2[:, 0:L]
                        .rearrange("c (k two) -> c k two", two=2)
                        .unsqueeze(1)
                        .broadcast_to([128, nj, L // 2, 2])
                    )
                    in1 = (
                        qp2[:, qoff + 2 * j0 : qoff + 2 * j1]
                        .rearrange("c (j two) -> c j two", two=2)
                        .unsqueeze(2)
                        .broadcast_to([128, nj, L // 2, 2])
                    )
                    s_out = S_all[:, j0 * L : j1 * L].rearrange(
                        "c (j k two) -> c j k two", two=2, k=L // 2
                    )
                    nc.vector.tensor_add(s_out, in0, in1)
                S_tanh = spool.tile([128, 32 * L], BF16, name=f"T{s}", tag="T")
                cut = 8 * L
                nc.scalar.activation(S_tanh[:, 0:cut], S_all[:, 0:cut], TANH)
                nc.scalar.activation(
                    S_tanh[:, cut : 32 * L], S_all[:, cut : 32 * L], TANH
                )

                out_ps = oup.tile([TQ, H + 1], F32, name=f"ops{s}", tag="ou")
                for ci, (off, w) in enumerate(chs):
                    kn = knat[(s, ci)]
                    sc_ps = scp.tile([128, TQ], F32, name=f"sc{s}_{ci}", tag="sc")
                    for j in range(32):
                        nc.tensor.matmul(
                            sc_ps[0:w, 2 * j : 2 * j + 2],
                            lhsT=S_tanh[:, j * L + off : j * L + off + w],
                            rhs=v2_sb,
                            start=True,
                            stop=True,
                        )
                    E = wpool.tile([128, TQ], F32, name=f"E{s}_{ci}", tag=f"E{ci}")
                    nc.scalar.activation(
                        E[0:w, :], sc_ps[0:w, :], EXP, bias=kn[0:w, H + 1 : H + 2]
                    )
                    nc.tensor.matmul(
                        out_ps,
                        lhsT=E[0:w, 0:TQ],
                        rhs=kn[0:w, 0 : H + 1],
                        start=(ci == 0),
                        stop=(ci == nch - 1),
                    )

                recip = wpool.tile([TQ, 1], F32, name=f"rc{s}", tag="rc")
                nc.vector.reciprocal(recip, out_ps[:, H : H + 1])
                out_sb = wpool.tile([TQ, H], F32, name=f"osb{s}", tag="osb")
                nc.vector.tensor_scalar_mul(out_sb, out_ps[:, 0:H], recip)
                nc.sync.dma_start(out=o_d[s], in_=out_sb)

    nc.compile()
    return nc


def _get_prog(L_slots):
    if L_slots not in _prog_cache:
        _prog_cache[L_slots] = _build(L_slots)
    return _prog_cache[L_slots]


def _plan(seq_len_flat):
    sl = np.asarray(seq_len_flat).reshape(-1).astype(np.int64)
    order = np.argsort(-sl, kind="stable")
    assign = np.zeros((NCORES, SLOTS), dtype=np.int64)
    L_slots = []
    for s in range(SLOTS):
        grp = order[NCORES * s : NCORES * (s + 1)]
        assign[:, s] = grp
        L = int(max(1, sl[grp].max()))
        L_slots.append(min(TK, _roundup(L, 8)))
    return tuple(L_slots), assign, sl


def _make_in_maps(queries, keys, sl, assign, W_q, W_k, v, b):
    vv = np.asarray(v, dtype=np.float32).reshape(-1)
    base = np.zeros((128, 390), np.float32)
    base[:, 130:258] = np.eye(128, dtype=np.float32)
    base[0:H, 258:322] = W_k
    base[0:H, 322:386] = W_q
    base[0:H, 386] = np.asarray(b, np.float32).reshape(-1)

    in_maps = []
    for c in range(NCORES):
        bidx = assign[c]
        cx = base.copy()
        cx[:, 0:64] = queries[bidx[0:2]].reshape(128, H)
        cx[:, 64:128] = queries[bidx[2:4]].reshape(128, H)
        cx[0:64, 128] = vv
        cx[64:128, 129] = vv
        keysx = np.zeros((SLOTS, TK, KX), np.float32)
        keysx[:, :, 0:H] = keys[bidx]
        keysx[:, :, H] = 1.0
        for s_i, b_i in enumerate(bidx):
            keysx[s_i, sl[b_i] :, H + 1] = MASK_NEG
        in_maps.append(
            {
                "keysx": keysx,
                "cx": cx,
            }
        )
    return in_maps


def _run_spmd(nc, in_maps, trace=False, trace_kwargs=None):
    from concourse.bass_interp import get_hw_module

    old = nc.m
    nc.m = get_hw_module(nc.m)
    try:
        res = bass_utils.run_bass_kernel_spmd(
            nc,
            in_maps,
            core_ids=list(range(NCORES)),
            trace=trace,
            **(trace_kwargs or {}),
        )
    finally:
        nc.m = old
    return res


def kernel(queries, keys, seq_len, W_q, W_k, v, b, _trace=False):
    queries = np.asarray(queries, dtype=np.float32)
    keys = np.asarray(keys, dtype=np.float32)
    L_slots, assign, sl = _plan(seq_len)
    nc = _get_prog(L_slots)
    in_maps = _make_in_maps(queries, keys, sl, assign, W_q, W_k, v, b)
    res = _run_spmd(nc, in_maps, trace=_trace)
    out = np.zeros((B, TQ, H), np.float32)
    for c in range(NCORES):
        o = res.results[c]["out"]
        for s_i, b_i in enumerate(assign[c]):
            out[b_i] = o[s_i]
    if _trace:
        kernel._last_results = res
    return out
